# revision 2
# baseline (speedup 1.0000x reference)
"""CrossAttention (channel attention) Trainium2 kernel.

Math (per batch element b):
    q = x Wq^T ; k = y Wk^T ; v = y Wv^T          (N=4096 tokens, C=1024 ch)
    per head h (H=16, D=64):
      scores_h = (Qh^T Kh) * D^-0.5 = Wq_h (x^T y) Wk_h^T * s   (D x D)
      attn_h = softmax(scores_h, axis=-1)
      z_h    = Vh attn_h^T                         (N x D)
    out = z Wp^T + bp

Reassociated (saves ~40% FLOPs and avoids transposing x):
    G   = y^T x                    (C x C)   contraction over n: natural layouts
    A   = G^T Wk^T                 (C x C)
    S_h = (s*Wq_h) A_h             (D x D)  -> softmax (unnormalized probs P_h,
                                              row sums r)
    M_h = P_h Wv_h                 (D x C);  Mall[ci, h*D+d] = M_h[d, ci]/r_d
    P   = Mall Wp^T                (C x C)
    out = y P + bp                 (N x C)

Sharding: pure data-parallel over batch B=8 across the 8 NeuronCores.
All on-chip matmuls run in fp16 (full PE rate) with fp32 PSUM accumulation.
"""

import os
import sys

import numpy as np

sys.path.insert(0, "/opt/trn_rl_repo")

import concourse.bass as bass  # noqa: E402
import concourse.mybir as mybir  # noqa: E402
import concourse.tile as tile  # noqa: E402
from concourse import bacc  # noqa: E402
from concourse.masks import make_identity  # noqa: E402

F16 = mybir.dt.float16
F32 = mybir.dt.float32
AX = mybir.AxisListType
AF = mybir.ActivationFunctionType

B, N, C, H = 8, 4096, 1024, 16
D = C // H          # 64
SCALE = D ** -0.5
NT = N // 128       # 32 n-tiles
CT = C // 128       # 8 channel tiles
PAIRS = H // 2      # 8 head pairs


def build_kernel():
    nc = bacc.Bacc("TRN2", target_bir_lowering=False)

    x_d = nc.dram_tensor("x", [N, C], F32, kind="ExternalInput")
    y_d = nc.dram_tensor("y", [N, C], F32, kind="ExternalInput")
    wqts_d = nc.dram_tensor("wqts", [C, C], F16, kind="ExternalInput")  # (Wq*s).T
    wkt_d = nc.dram_tensor("wkt", [C, C], F16, kind="ExternalInput")    # Wk.T
    wv_d = nc.dram_tensor("wv", [C, C], F16, kind="ExternalInput")      # Wv
    wpt_d = nc.dram_tensor("wpt", [C, C], F16, kind="ExternalInput")    # Wp.T
    bp_d = nc.dram_tensor("bp", [C], F32, kind="ExternalInput")
    out_d = nc.dram_tensor("out", [N, C], F32, kind="ExternalOutput")

    with tile.TileContext(nc) as tc:
        with (
            tc.tile_pool(name="persist", bufs=1) as persist,
            tc.tile_pool(name="stream", bufs=3) as stream,
            tc.tile_pool(name="small", bufs=4) as small,
            tc.tile_pool(name="dram", bufs=1, space="DRAM") as drampool,
        ):
            # ---- constants / weights ----------------------------------
            wqts = persist.tile([128, CT, C], F16, name="wqts_sb")
            wkt = persist.tile([128, CT, C], F16, name="wkt_sb")
            wv = persist.tile([128, CT, C], F16, name="wv_sb")
            wpt = persist.tile([128, CT, C], F16, name="wpt_sb")
            for sb, dr in ((wqts, wqts_d), (wkt, wkt_d), (wv, wv_d), (wpt, wpt_d)):
                nc.sync.dma_start(sb, dr[:].rearrange("(t p) c -> p t c", p=128))

            bias = persist.tile([128, C], F32, name="bias_sb")
            bp_ap = bp_d[:]
            nc.sync.dma_start(
                bias,
                bass.AP(tensor=bp_ap.tensor, offset=bp_ap.offset,
                        ap=[[0, 128]] + list(bp_ap.ap)),
            )

            id128 = persist.tile([128, 128], F16, name="id128")
            make_identity(nc, id128)
            # identity block living on partitions 64..127: idhi[64+i, i] = 1
            idhi = persist.tile([128, D], F16, name="idhi")
            nc.gpsimd.memset(idhi, 0.0)
            nc.gpsimd.affine_select(
                out=idhi, in_=idhi,
                compare_op=mybir.AluOpType.not_equal,
                fill=1.0, base=-D, pattern=[[-1, D]], channel_multiplier=1,
            )

            # big shared slot: y16 (phases 1-2,) later reused as ytall (phase 7)
            y16 = persist.tile([128, NT, C], F16, name="y16", tag="ybig")
            g2 = persist.tile([128, CT, C], F16, name="g2_sb", tag="sc1")
            y16_dram = drampool.tile([N, C], F16, name="y16_dram")

            # ================= phase 1+2: G = y^T x =====================
            # psum holds all 8 cj tiles x 512 ci (one ci-half per pass)
            with tc.tile_pool(name="ps_g2", bufs=1, space="PSUM") as ps_g2_pool:
                for p_half in range(2):
                    cisl = slice(p_half * 512, (p_half + 1) * 512)
                    ps8 = ps_g2_pool.tile([128, CT, 512], F32, name="ps_g2")
                    for nt in range(NT):
                        if p_half == 0:
                            yst = stream.tile([128, C], F32, name="yst", tag="yst")
                            nc.sync.dma_start(yst, y_d[nt * 128:(nt + 1) * 128, :])
                            nc.scalar.copy(out=y16[:, nt, :], in_=yst)
                            nc.sync.dma_start(
                                y16_dram[nt * 128:(nt + 1) * 128, :], y16[:, nt, :]
                            )
                        xst = stream.tile([128, 512], F32, name="xst", tag="xst")
                        nc.sync.dma_start(xst, x_d[nt * 128:(nt + 1) * 128, cisl])
                        x16 = stream.tile([128, 512], F16, name="x16", tag="x16")
                        nc.scalar.copy(out=x16, in_=xst)
                        for cj in range(CT):
                            nc.tensor.matmul(
                                ps8[:, cj, :],
                                lhsT=y16[:, nt, cj * 128:(cj + 1) * 128],
                                rhs=x16,
                                start=(nt == 0), stop=(nt == NT - 1),
                            )
                    for cj in range(CT):
                        nc.vector.tensor_copy(out=g2[:, cj, cisl], in_=ps8[:, cj, :])

            # ================= phase 3: A = G^T Wk^T ====================
            a_sb = persist.tile([128, CT, C], F16, name="a_sb", tag="sc2")
            with tc.tile_pool(name="ps_a", bufs=2, space="PSUM") as ps_a_pool:
                for ci in range(CT):
                    psa = ps_a_pool.tile([128, C], F32, name="ps_a")
                    for cj in range(CT):
                        for ch in range(2):
                            nc.tensor.matmul(
                                psa[:, ch * 512:(ch + 1) * 512],
                                lhsT=g2[:, cj, ci * 128:(ci + 1) * 128],
                                rhs=wkt[:, cj, ch * 512:(ch + 1) * 512],
                                start=(cj == 0), stop=(cj == CT - 1),
                            )
                    nc.vector.tensor_copy(out=a_sb[:, ci, :], in_=psa)

            # ====== phase 4+5: scores -> softmax -> Mall^T ==============
            mallT = persist.tile([128, CT, C], F16, name="mallT", tag="sc1")
            with (
                tc.tile_pool(name="ps_s", bufs=2, space="PSUM") as ps_s_pool,
                tc.tile_pool(name="ps_t", bufs=2, space="PSUM") as ps_t_pool,
                tc.tile_pool(name="ps_m", bufs=2, space="PSUM") as ps_m_pool,
            ):
                for t in range(PAIRS):
                    ps_s = ps_s_pool.tile([128, D], F32, name="ps_s")
                    for h2 in range(2):
                        h = 2 * t + h2
                        hsl = slice(h * D, (h + 1) * D)
                        for ci in range(CT):
                            nc.tensor.matmul(
                                ps_s[h2 * D:(h2 + 1) * D, :],
                                lhsT=wqts[:, ci, hsl],
                                rhs=a_sb[:, ci, hsl],
                                start=(ci == 0), stop=(ci == CT - 1),
                            )
                    mx = small.tile([128, 1], F32, name="mx")
                    nc.vector.reduce_max(out=mx, in_=ps_s, axis=AX.X, negate=True)
                    probs = small.tile([128, D], F16, name="probs")
                    sumex = small.tile([128, 1], F32, name="sumex")
                    nc.scalar.activation(
                        out=probs, in_=ps_s, func=AF.Exp,
                        bias=mx, scale=1.0, accum_out=sumex,
                    )
                    rcp = small.tile([128, 1], F32, name="rcp")
                    nc.vector.reciprocal(out=rcp, in_=sumex)

                    at_ps = ps_t_pool.tile([128, D], F16, name="at_ps")
                    nc.tensor.transpose(at_ps[0:D, :], probs[0:D, :],
                                        id128[0:D, 0:D])
                    nc.tensor.transpose(at_ps[D:128, :], probs[D:128, :],
                                        idhi[D:128, :])
                    attnT = small.tile([128, D], F16, name="attnT")
                    nc.vector.tensor_copy(out=attnT, in_=at_ps)

                    for ch in range(2):
                        csl = slice(ch * 512, (ch + 1) * 512)
                        ps_m = ps_m_pool.tile([128, 512], F32, name="ps_m")
                        nc.tensor.matmul(ps_m[0:D, :], lhsT=attnT[0:D, :],
                                         rhs=wv[0:D, t, csl],
                                         start=True, stop=True)
                        nc.tensor.matmul(ps_m[D:128, :], lhsT=attnT[D:128, :],
                                         rhs=wv[D:128, t, csl],
                                         start=True, stop=True)
                        nc.vector.tensor_scalar_mul(
                            out=mallT[:, t, csl], in0=ps_m, scalar1=rcp,
                        )

            # ================= phase 6: P = Mall Wp^T ===================
            p_sb = persist.tile([128, CT, C], F16, name="p_sb", tag="sc2")
            with tc.tile_pool(name="ps_p", bufs=2, space="PSUM") as ps_p_pool:
                for ci in range(CT):
                    psp = ps_p_pool.tile([128, C], F32, name="ps_p")
                    for cp in range(CT):
                        for ch in range(2):
                            nc.tensor.matmul(
                                psp[:, ch * 512:(ch + 1) * 512],
                                lhsT=mallT[:, cp, ci * 128:(ci + 1) * 128],
                                rhs=wpt[:, cp, ch * 512:(ch + 1) * 512],
                                start=(cp == 0), stop=(cp == CT - 1),
                            )
                    nc.vector.tensor_copy(out=p_sb[:, ci, :], in_=psp)

            # ================= phase 7: out = y P + bp ==================
            # y^T tiles via DMA transpose of the fp16 copy of y in DRAM;
            # reuses the SBUF slot of y16 (same tag), which is dead by now.
            ytall = persist.tile([128, CT, N], F16, name="ytall", tag="ybig")
            for k in range(CT):
                nc.sync.dma_start_transpose(
                    ytall[:, k, :], y16_dram[:, k * 128:(k + 1) * 128]
                )
            with tc.tile_pool(name="ps_f", bufs=3, space="PSUM") as ps_f_pool:
                for nt in range(NT):
                    psf = ps_f_pool.tile([128, C], F32, name="ps_f")
                    for k in range(CT):
                        for ch in range(2):
                            nc.tensor.matmul(
                                psf[:, ch * 512:(ch + 1) * 512],
                                lhsT=ytall[:, k, nt * 128:(nt + 1) * 128],
                                rhs=p_sb[:, k, ch * 512:(ch + 1) * 512],
                                start=(k == 0), stop=(k == CT - 1),
                            )
                    osb = stream.tile([128, C], F32, name="osb", tag="osb")
                    nc.vector.tensor_add(out=osb, in0=psf, in1=bias)
                    nc.sync.dma_start(out_d[nt * 128:(nt + 1) * 128, :], osb)

    nc.compile()
    return nc


_NC_CACHE = None


def _get_nc():
    global _NC_CACHE
    if _NC_CACHE is None:
        _NC_CACHE = build_kernel()
    return _NC_CACHE


def run(inputs, trace=False, **kw):
    from concourse.bass_utils import run_bass_kernel_spmd

    x = np.asarray(inputs["x"], dtype=np.float32)
    y = np.asarray(inputs["y"], dtype=np.float32)
    Wq = np.asarray(inputs["Wq"], dtype=np.float32)
    Wk = np.asarray(inputs["Wk"], dtype=np.float32)
    Wv = np.asarray(inputs["Wv"], dtype=np.float32)
    Wp = np.asarray(inputs["Wp"], dtype=np.float32)
    bp = np.asarray(inputs["bp"], dtype=np.float32)

    wqts = np.ascontiguousarray((Wq.T * np.float32(SCALE)).astype(np.float16))
    wkt = np.ascontiguousarray(Wk.T.astype(np.float16))
    wv16 = np.ascontiguousarray(Wv.astype(np.float16))
    wpt = np.ascontiguousarray(Wp.T.astype(np.float16))

    nc = _get_nc()
    in_maps = [
        {
            "x": np.ascontiguousarray(x[b]),
            "y": np.ascontiguousarray(y[b]),
            "wqts": wqts,
            "wkt": wkt,
            "wv": wv16,
            "wpt": wpt,
            "bp": bp,
        }
        for b in range(B)
    ]
    res = run_bass_kernel_spmd(nc, in_maps, core_ids=list(range(B)),
                               trace=trace, **kw)
    out = np.stack([res.results[b]["out"] for b in range(B)], axis=0)
    return out, res


def kernel(**inputs) -> np.ndarray:
    out, _ = run(inputs)
    return out


if __name__ == "__main__":
    nc = build_kernel()
    print("build ok")


# revision 8
# speedup vs baseline: 1.2032x; 1.2032x over previous
"""CrossAttention (channel attention) Trainium2 kernel.

Math (per batch element b):
    q = x Wq^T ; k = y Wk^T ; v = y Wv^T          (N=4096 tokens, C=1024 ch)
    per head h (H=16, D=64):
      scores_h = (Qh^T Kh) * D^-0.5 = Wq_h (x^T y) Wk_h^T * s   (D x D)
      attn_h = softmax(scores_h, axis=-1)
      z_h    = Vh attn_h^T                         (N x D)
    out = z Wp^T + bp

Reassociated (saves ~40% FLOPs and avoids transposing x):
    G   = y^T x                    (C x C)   contraction over n: natural layouts
    A   = G^T Wk^T                 (C x C)
    S_h = (s*Wq_h) A_h             (D x D)  -> softmax (unnormalized probs P_h,
                                              row sums r)
    M_h = P_h Wv_h                 (D x C);  Mall[ci, h*D+d] = M_h[d, ci]/r_d
    P   = Mall Wp^T                (C x C)
    out = y P + bp                 (N x C)

Sharding: pure data-parallel over batch B=8 across the 8 NeuronCores.
All on-chip matmuls run in fp16 (full PE rate) with fp32 PSUM accumulation.
"""

import os
import sys

import numpy as np

sys.path.insert(0, "/opt/trn_rl_repo")

import concourse.bass as bass  # noqa: E402
import concourse.mybir as mybir  # noqa: E402
import concourse.tile as tile  # noqa: E402
from concourse import bacc  # noqa: E402
from concourse.masks import make_identity  # noqa: E402

F16 = mybir.dt.float16
F32 = mybir.dt.float32
AX = mybir.AxisListType
AF = mybir.ActivationFunctionType

B, N, C, H = 8, 4096, 1024, 16
D = C // H          # 64
SCALE = D ** -0.5
NT = N // 128       # 32 n-tiles
CT = C // 128       # 8 channel tiles
PAIRS = H // 2      # 8 head pairs


def build_kernel():
    nc = bacc.Bacc("TRN2", target_bir_lowering=False)

    x_d = nc.dram_tensor("x", [N, C], F32, kind="ExternalInput")
    y_d = nc.dram_tensor("y", [N, C], F32, kind="ExternalInput")
    wqts_d = nc.dram_tensor("wqts", [C, C], F16, kind="ExternalInput")  # (Wq*s).T
    wkt_d = nc.dram_tensor("wkt", [C, C], F16, kind="ExternalInput")    # Wk.T
    wv_d = nc.dram_tensor("wv", [C, C], F16, kind="ExternalInput")      # Wv
    wpt_d = nc.dram_tensor("wpt", [C, C], F16, kind="ExternalInput")    # Wp.T
    bp_d = nc.dram_tensor("bp", [C], F32, kind="ExternalInput")
    out_d = nc.dram_tensor("out", [N, C], F32, kind="ExternalOutput")

    with tile.TileContext(nc) as tc:
        with (
            tc.tile_pool(name="persist", bufs=1) as persist,
            tc.tile_pool(name="stream", bufs=4) as stream,
            tc.tile_pool(name="small", bufs=4) as small,
            tc.tile_pool(name="dram", bufs=1, space="DRAM") as drampool,
        ):
            # big shared slot: y16 (phases 1-2,) later reused as ytall (phase 7)
            y16 = persist.tile([128, NT, C], F16, name="y16", tag="ybig")
            g2 = persist.tile([128, CT, C], F16, name="g2_sb", tag="sc1")
            y16_dram = drampool.tile([N, C], F16, name="y16_dram")

            # ================= phase 1+2: G = y^T x =====================
            # psum holds all 8 cj tiles x 512 ci (one ci-half per pass).
            # The fp16 copy of y is written back to DRAM during pass 2 (pass 1
            # is already at the DMA-bandwidth limit with x+y inbound).
            with tc.tile_pool(name="ps_g2", bufs=1, space="PSUM") as ps_g2_pool:
                for p_half in range(2):
                    cisl = slice(p_half * 512, (p_half + 1) * 512)
                    ps8 = ps_g2_pool.tile([128, CT, 512], F32, name="ps_g2")
                    for nt in range(NT):
                        if p_half == 0:
                            yst = stream.tile([128, C], F32, name="yst", tag="yst", bufs=3)
                            nc.sync.dma_start(yst, y_d[nt * 128:(nt + 1) * 128, :])
                            nc.scalar.copy(out=y16[:, nt, :], in_=yst)
                        else:
                            nc.gpsimd.dma_start(
                                y16_dram[nt * 128:(nt + 1) * 128, :], y16[:, nt, :]
                            )
                        xst = stream.tile([128, 512], F32, name="xst", tag="xst")
                        nc.sync.dma_start(xst, x_d[nt * 128:(nt + 1) * 128, cisl])
                        x16 = stream.tile([128, 512], F16, name="x16", tag="x16")
                        nc.scalar.copy(out=x16, in_=xst)
                        for cj in range(CT):
                            nc.tensor.matmul(
                                ps8[:, cj, :],
                                lhsT=y16[:, nt, cj * 128:(cj + 1) * 128],
                                rhs=x16,
                                start=(nt == 0), stop=(nt == NT - 1),
                            )
                    for cj in range(CT):
                        nc.vector.tensor_copy(out=g2[:, cj, cisl], in_=ps8[:, cj, :])

            # y^T tiles for phase 7 via DMA transpose of the fp16 copy of y.
            # Dispatched on gpsimd queues so they don't contend with the sync
            # queue; they run during phases 3-6.
            ytall = persist.tile([128, CT, N], F16, name="ytall", tag="ybig")
            for k in range(CT):
                nc.scalar.dma_start_transpose(
                    ytall[:, k, :], y16_dram[:, k * 128:(k + 1) * 128]
                )

            # ---- constants / weights (needed from phase 3 on) ----------
            wqts = persist.tile([128, CT, C], F16, name="wqts_sb")
            wkt = persist.tile([128, CT, C], F16, name="wkt_sb")
            wv = persist.tile([128, CT, C], F16, name="wv_sb")
            wpt = persist.tile([128, CT, C], F16, name="wpt_sb")
            for sb, dr in ((wkt, wkt_d), (wqts, wqts_d), (wv, wv_d), (wpt, wpt_d)):
                nc.sync.dma_start(sb, dr[:].rearrange("(t p) c -> p t c", p=128))

            bias = persist.tile([128, C], F32, name="bias_sb")
            bp_ap = bp_d[:]
            nc.sync.dma_start(
                bias,
                bass.AP(tensor=bp_ap.tensor, offset=bp_ap.offset,
                        ap=[[0, 128]] + list(bp_ap.ap)),
            )

            id128 = persist.tile([128, 128], F16, name="id128")
            make_identity(nc, id128)
            # identity block living on partitions 64..127: idhi[64+i, i] = 1
            idhi = persist.tile([128, D], F16, name="idhi")
            nc.gpsimd.memset(idhi, 0.0)
            nc.gpsimd.affine_select(
                out=idhi, in_=idhi,
                compare_op=mybir.AluOpType.not_equal,
                fill=1.0, base=-D, pattern=[[-1, D]], channel_multiplier=1,
            )

            # ================= phase 3: A = G^T Wk^T ====================
            a_sb = persist.tile([128, CT, C], F16, name="a_sb", tag="sc2")
            with tc.tile_pool(name="ps_a", bufs=2, space="PSUM") as ps_a_pool:
                for ci in range(CT):
                    psa = ps_a_pool.tile([128, C], F32, name="ps_a")
                    for cj in range(CT):
                        for ch in range(2):
                            nc.tensor.matmul(
                                psa[:, ch * 512:(ch + 1) * 512],
                                lhsT=g2[:, cj, ci * 128:(ci + 1) * 128],
                                rhs=wkt[:, cj, ch * 512:(ch + 1) * 512],
                                start=(cj == 0), stop=(cj == CT - 1),
                            )
                    nc.vector.tensor_copy(out=a_sb[:, ci, :], in_=psa)

            # ====== phase 4+5: scores -> softmax -> Mall^T ==============
            mallT = persist.tile([128, CT, C], F16, name="mallT", tag="sc1")
            with (
                tc.tile_pool(name="ps_s", bufs=3, space="PSUM") as ps_s_pool,
                tc.tile_pool(name="ps_t", bufs=2, space="PSUM") as ps_t_pool,
                tc.tile_pool(name="ps_m", bufs=2, space="PSUM") as ps_m_pool,
            ):
                for t in range(PAIRS):
                    ps_s = ps_s_pool.tile([128, D], F32, name="ps_s")
                    for h2 in range(2):
                        h = 2 * t + h2
                        hsl = slice(h * D, (h + 1) * D)
                        for ci in range(CT):
                            nc.tensor.matmul(
                                ps_s[h2 * D:(h2 + 1) * D, :],
                                lhsT=wqts[:, ci, hsl],
                                rhs=a_sb[:, ci, hsl],
                                start=(ci == 0), stop=(ci == CT - 1),
                            )
                    mx = small.tile([128, 1], F32, name="mx")
                    nc.vector.reduce_max(out=mx, in_=ps_s, axis=AX.X, negate=True)
                    probs = small.tile([128, D], F16, name="probs")
                    sumex = small.tile([128, 1], F32, name="sumex")
                    nc.scalar.activation(
                        out=probs, in_=ps_s, func=AF.Exp,
                        bias=mx, scale=1.0, accum_out=sumex,
                    )
                    rcp = small.tile([128, 1], F32, name="rcp")
                    nc.vector.reciprocal(out=rcp, in_=sumex)

                    at_ps = ps_t_pool.tile([128, D], F16, name="at_ps")
                    nc.tensor.transpose(at_ps[0:D, :], probs[0:D, :],
                                        id128[0:D, 0:D])
                    nc.tensor.transpose(at_ps[D:128, :], probs[D:128, :],
                                        idhi[D:128, :])
                    attnT = small.tile([128, D], F16, name="attnT")
                    nc.vector.tensor_copy(out=attnT, in_=at_ps)

                    for ch in range(2):
                        csl = slice(ch * 512, (ch + 1) * 512)
                        ps_m = ps_m_pool.tile([128, 512], F32, name="ps_m")
                        nc.tensor.matmul(ps_m[0:D, :], lhsT=attnT[0:D, :],
                                         rhs=wv[0:D, t, csl],
                                         start=True, stop=True)
                        nc.tensor.matmul(ps_m[D:128, :], lhsT=attnT[D:128, :],
                                         rhs=wv[D:128, t, csl],
                                         start=True, stop=True)
                        nc.vector.tensor_scalar_mul(
                            out=mallT[:, t, csl], in0=ps_m, scalar1=rcp,
                        )

            # ================= phase 6: P = Mall Wp^T ===================
            p_sb = persist.tile([128, CT, C], F16, name="p_sb", tag="sc2")
            with tc.tile_pool(name="ps_p", bufs=2, space="PSUM") as ps_p_pool:
                for ci in range(CT):
                    psp = ps_p_pool.tile([128, C], F32, name="ps_p")
                    for cp in range(CT):
                        for ch in range(2):
                            nc.tensor.matmul(
                                psp[:, ch * 512:(ch + 1) * 512],
                                lhsT=mallT[:, cp, ci * 128:(ci + 1) * 128],
                                rhs=wpt[:, cp, ch * 512:(ch + 1) * 512],
                                start=(cp == 0), stop=(cp == CT - 1),
                            )
                    nc.vector.tensor_copy(out=p_sb[:, ci, :], in_=psp)

            # ================= phase 7: out = y P + bp ==================
            with tc.tile_pool(name="ps_f", bufs=3, space="PSUM") as ps_f_pool:
                for nt in range(NT):
                    psf = ps_f_pool.tile([128, C], F32, name="ps_f")
                    for k in range(CT):
                        for ch in range(2):
                            nc.tensor.matmul(
                                psf[:, ch * 512:(ch + 1) * 512],
                                lhsT=ytall[:, k, nt * 128:(nt + 1) * 128],
                                rhs=p_sb[:, k, ch * 512:(ch + 1) * 512],
                                start=(k == 0), stop=(k == CT - 1),
                            )
                    osb = stream.tile([128, C], F32, name="osb", tag="osb", bufs=3)
                    nc.vector.tensor_add(out=osb, in0=psf, in1=bias)
                    nc.sync.dma_start(out_d[nt * 128:(nt + 1) * 128, :], osb)

    nc.compile()
    return nc


_NC_CACHE = None


def _get_nc():
    global _NC_CACHE
    if _NC_CACHE is None:
        _NC_CACHE = build_kernel()
    return _NC_CACHE


def run(inputs, trace=False, **kw):
    from concourse.bass_utils import run_bass_kernel_spmd

    x = np.asarray(inputs["x"], dtype=np.float32)
    y = np.asarray(inputs["y"], dtype=np.float32)
    Wq = np.asarray(inputs["Wq"], dtype=np.float32)
    Wk = np.asarray(inputs["Wk"], dtype=np.float32)
    Wv = np.asarray(inputs["Wv"], dtype=np.float32)
    Wp = np.asarray(inputs["Wp"], dtype=np.float32)
    bp = np.asarray(inputs["bp"], dtype=np.float32)

    wqts = np.ascontiguousarray((Wq.T * np.float32(SCALE)).astype(np.float16))
    wkt = np.ascontiguousarray(Wk.T.astype(np.float16))
    wv16 = np.ascontiguousarray(Wv.astype(np.float16))
    wpt = np.ascontiguousarray(Wp.T.astype(np.float16))

    nc = _get_nc()
    in_maps = [
        {
            "x": np.ascontiguousarray(x[b]),
            "y": np.ascontiguousarray(y[b]),
            "wqts": wqts,
            "wkt": wkt,
            "wv": wv16,
            "wpt": wpt,
            "bp": bp,
        }
        for b in range(B)
    ]
    res = run_bass_kernel_spmd(nc, in_maps, core_ids=list(range(B)),
                               trace=trace, **kw)
    out = np.stack([res.results[b]["out"] for b in range(B)], axis=0)
    return out, res


def kernel(**inputs) -> np.ndarray:
    out, _ = run(inputs)
    return out


if __name__ == "__main__":
    nc = build_kernel()
    print("build ok")
